# revision 17
# baseline (speedup 1.0000x reference)
"""ChineseCLIPVisionLayer on 8 trn2 NeuronCores.

Sharding: pure data-parallel over batch (B=32 -> 4 per core), zero
collectives. Weights are host-repacked into tile-contiguous layouts and
replicated to every core.

Per-core pipeline (activations feeding matmuls live transposed [D, S] so
the contraction dim sits on SBUF partitions):
  LN1 -> PE-transpose -> h^T ; q^T,k^T (transposed), v (natural)
  attention with keys zero-padded to 384 = 3x128 so every PE instruction
  is a full 128-row/col shape; the two heads of each 128-partition chunk
  are packed via PE row/col tiling (auto tile_position). Scores for one
  head land in a 3-bank PSUM tile; one strided Exp covers all 3 chunks.
  Pad-key rows are killed by a K=1 matmul adding -60 pre-exp (exp -> 0
  in f16). Softmax denominators come out of the ones-matmul already
  broadcast across the head's 64 partitions (M=64 all-ones stationary),
  so a single reciprocal feeds the fused normalize-evict of attn^T.
  v_b is folded into o_b on host.
  QKV projections of batch b+1 (and out_proj of earlier batches) are
  interleaved into attention of batch b as dense 128x128 filler matmuls:
  they absorb the exp latency and keep the PE clock-gate (HAM) warm.
  out_proj -> transpose + residual -> x1 (f16, SBUF-resident) -> LN2 ->
  h2^T ; MLP single pass (each weight tile streamed once, contiguous);
  quick-gelu == Gelu_apprx_sigmoid; fc2 out -> transpose + residual.
"""

from collections import deque
from contextlib import ExitStack

import numpy as np

import concourse.bass as bass
import concourse.mybir as mybir
import concourse.tile as tile
from concourse import bacc, bass_utils
from concourse.masks import make_identity

N_CORES = 8
B, S, D = 32, 257, 1024
H, HD = 16, 64
FF = 4096
EPS = 1e-5
SCALE = HD ** -0.5
NB = B // N_CORES

F32 = mybir.dt.float32
F16 = mybir.dt.float16
AF = mybir.ActivationFunctionType
ALU = mybir.AluOpType

SQ = [(0, 128), (128, 128), (256, 1)]  # seq chunks (partition tiling)
SE = 258    # query free-dim padded length
SKP = 384   # key length padded to 3x128
DC = D // 128
FC = FF // 128


def build():
    nc = bacc.Bacc("TRN2", target_bir_lowering=False, debug=False,
                   num_devices=N_CORES)

    def din(name, shape, dt=F32):
        return nc.dram_tensor(name, shape, dt, kind="ExternalInput").ap()

    x_d = din("x", [NB, S, D])
    qw_d = din("qw", [128, DC, D], F16)
    kw_d = din("kw", [128, DC, D], F16)
    vw_d = din("vw", [128, DC, D], F16)
    ow_d = din("ow", [128, DC, D], F16)
    f1w_d = din("f1w", [128, FC, DC, 128], F16)
    f2w_d = din("f2w", [128, DC, FC, 128], F16)
    qb_d = din("qb", [D])
    kb_d = din("kb", [D])
    ob_d = din("ob", [D])
    f1b_d = din("f1b", [FF])
    f2b_d = din("f2b", [D])
    g1_d = din("g1", [D])
    b1_d = din("b1", [D])
    g2_d = din("g2", [D])
    b2_d = din("b2", [D])
    out_d = nc.dram_tensor("out", [NB, S, D], F32, kind="ExternalOutput").ap()

    with tile.TileContext(nc) as tc:
        with ExitStack() as es:
            P = lambda name, bufs, **kw: es.enter_context(
                tc.tile_pool(name=name, bufs=bufs, **kw))
            const = P("const", 1)
            biasp = P("bias", 1)
            xio = P("xio", 3)
            stat = P("stat", 8)

            ident = const.tile([128, 128], F32)
            make_identity(nc, ident)
            ident16 = const.tile([128, 128], F16)
            make_identity(nc, ident16)
            onesb = const.tile([128, 64], F16)
            nc.vector.memset(onesb[:], 1.0)
            ones_row = const.tile([1, SE], F16)
            nc.vector.memset(ones_row[:], 1.0)
            padneg = const.tile([1, 128], F16)
            nc.vector.memset(padneg[:], -60.0)
            nc.vector.memset(padneg[:, 0:1], 0.0)
            epsc = const.tile([128, 1], F32)
            nc.vector.memset(epsc[:], EPS)

            def load_bias(dram, n):
                t = biasp.tile([128, n // 128], F32, name=f"bias_{dram.name}")
                nc.sync.dma_start(t[:], dram.rearrange("(c p) -> p c", p=128))
                return t

            qb_sb = load_bias(qb_d, D)
            kb_sb = load_bias(kb_d, D)
            ob_sb = load_bias(ob_d, D)
            f1b_sb = load_bias(f1b_d, FF)
            f2b_sb = load_bias(f2b_d, D)
            g1_sb = load_bias(g1_d, D)
            b1_sb = load_bias(b1_d, D)
            g2_sb = load_bias(g2_d, D)
            b2_sb = load_bias(b2_d, D)

            def layer_norm(src_tiles, hpool):
                """src_tiles: 3 natural tiles [(pz, D)]; returns
                (x-mu)*rstd tiles (gamma/beta applied at transpose).
"""
                out_tiles = []
                for j, (o, pz) in enumerate(SQ):
                    xt = src_tiles[j]
                    st = stat.tile([pz, 2, 6], F32, name="st", tag="st")
                    nc.vector.bn_stats(st[:, 0, :], xt[:, 0:512])
                    nc.vector.bn_stats(st[:, 1, :], xt[:, 512:1024])
                    mv = stat.tile([pz, 2], F32, name="mv", tag="mv")
                    nc.vector.bn_aggr(mv[:], st[:])
                    rstd = stat.tile([pz, 1], F32, name="rstd", tag="rstd")
                    nc.scalar.activation(rstd[:], mv[:, 1:2], AF.Sqrt,
                                         bias=epsc[:pz, :])
                    nc.vector.reciprocal(rstd[:], rstd[:])
                    ht = hpool.tile([pz, D], F32, name="hn", tag="hn")
                    nc.vector.tensor_scalar(
                        out=ht[:], in0=xt[:], scalar1=mv[:, 0:1],
                        scalar2=rstd[:], op0=ALU.subtract, op1=ALU.mult)
                    out_tiles.append(ht)
                return out_tiles

            def transpose_to_T(nat_tiles, dst_pool, g_sb, bt_sb, tag, pspool):
                outs = []
                for dc in range(DC):
                    ps = pspool.tile([128, SE], F32, name="psT", tag="pp")
                    for j, (o, pz) in enumerate(SQ):
                        nc.tensor.transpose(
                            ps[:, o:o + pz],
                            nat_tiles[j][:, dc * 128:(dc + 1) * 128],
                            ident[:pz, :pz])
                    t = dst_pool.tile([128, SE], F16, name=f"{tag}", tag=tag)
                    nc.vector.tensor_scalar(
                        out=t[:], in0=ps[:], scalar1=g_sb[:, dc:dc + 1],
                        scalar2=bt_sb[:, dc:dc + 1], op0=ALU.mult, op1=ALU.add)
                    outs.append(t)
                return outs

            # ---------- stage A: load x, LN1, h^T ----------
            esPS = ExitStack()
            psABC = esPS.enter_context(
                tc.tile_pool(name="psABC", bufs=3, space="PSUM"))
            esW = ExitStack()
            wP = esW.enter_context(tc.tile_pool(name="wP", bufs=3))

            def load_w(dram):
                wt = wP.tile([128, DC, D], F16, name="pw", tag="pw")
                nc.sync.dma_start(wt[:], dram)
                return wt

            esCD = ExitStack()
            attnTp = esCD.enter_context(
                tc.tile_pool(name="attnT", bufs=NB * DC, side="right"))
            esA_HT = ExitStack()
            HTp = esA_HT.enter_context(
                tc.tile_pool(name="HT", bufs=NB * DC, side="right"))
            esA = ExitStack()
            hnat = esA.enter_context(
                tc.tile_pool(name="hnat", bufs=3, side="right"))
            HT = []
            w_loads = {}

            def stage_a(b):
                xts = []
                for j, (o, pz) in enumerate(SQ):
                    xt = xio.tile([pz, D], F32, name="xin", tag="xin")
                    nc.sync.dma_start(xt[:, 0:512], x_d[b, o:o + pz, 0:512])
                    nc.sync.dma_start(xt[:, 512:1024],
                                      x_d[b, o:o + pz, 512:1024])
                    xts.append(xt)
                hts = layer_norm(xts, hnat)
                HT.append(transpose_to_T(hts, HTp, g1_sb, b1_sb, "HT", psABC))

            # ---------- stages B+C fused ----------
            esD2 = ExitStack()
            aoTp = esD2.enter_context(tc.tile_pool(name="aoT", bufs=NB * DC))
            esBC = ExitStack()
            qTp = esBC.enter_context(tc.tile_pool(name="qT", bufs=NB * DC))
            kTp = esBC.enter_context(tc.tile_pool(name="kT", bufs=NB * DC))
            vp = esBC.enter_context(tc.tile_pool(name="vna", bufs=NB * 3))

            qT = [[None] * DC for _ in range(NB)]
            kT = [[None] * DC for _ in range(NB)]
            vna = [None] * NB
            aoT = [[None] * DC for _ in range(NB)]
            attnT = [[None] * DC for _ in range(NB)]

            def q_mc(b, mc, wt, w_sb, bias_sb, outs, tag, kpad):
                ps = psABC.tile([128, SE], F32, name="psP", tag="pp")
                for kc in range(DC):
                    nc.tensor.matmul(
                        ps[:], w_sb[:, kc, mc * 128:(mc + 1) * 128],
                        HT[b][kc][:], start=(kc == 0), stop=(kc == DC - 1))
                w = SKP if kpad else SE
                t = wt.tile([128, w], F16, name=tag, tag=tag)
                nc.vector.tensor_scalar_add(
                    t[:, 0:SE], ps[:], bias_sb[:, mc:mc + 1])
                if kpad:
                    nc.vector.memset(t[:, SE - 1:SKP], 0.0)
                outs[b][mc] = t

            def o_mc(b, mc, pspool=None):
                ps = (pspool or psABC).tile([128, SE], F32, name="psP",
                                            tag="pp")
                for kc in range(DC):
                    nc.tensor.matmul(
                        ps[:], w_loads["o"][:, kc, mc * 128:(mc + 1) * 128],
                        attnT[b][kc][:], start=(kc == 0), stop=(kc == DC - 1))
                t = aoTp.tile([128, SE], F16, name="aoT", tag="aoT")
                nc.vector.tensor_scalar_add(t[:], ps[:], ob_sb[:, mc:mc + 1])
                aoT[b][mc] = t

            def v_grp(b, j, half):
                o, pz = SQ[j]
                ps = psABC.tile([128, 512], F32, name="psV", tag="pp")
                for kc in range(DC):
                    nc.tensor.matmul(
                        ps[:pz, :], HT[b][kc][:, o:o + pz],
                        w_loads["v"][:, kc, half * 512:(half + 1) * 512],
                        start=(kc == 0), stop=(kc == DC - 1))
                nc.vector.tensor_copy(
                    vna[b][j][0:pz, half * 512:(half + 1) * 512], ps[:pz, :])

            def proj_tasks(b):
                tasks = []
                for mc in range(DC):
                    tasks.append(lambda b=b, mc=mc: q_mc(
                        b, mc, qTp, w_loads["q"], qb_sb, qT, "qT", False))
                    tasks.append(lambda b=b, mc=mc: q_mc(
                        b, mc, kTp, w_loads["k"], kb_sb, kT, "kT", True))

                def v_alloc(b=b):
                    vna[b] = [vp.tile([128, D], F16, name="vna", tag="vna")
                              for _ in SQ]
                    nc.vector.memset(vna[b][2][:], 0.0)
                    v_grp(b, 0, 0)
                tasks.append(v_alloc)
                for (j, half) in [(0, 1), (1, 0), (1, 1), (2, 0), (2, 1)]:
                    tasks.append(lambda b=b, j=j, half=half: v_grp(b, j, half))
                return tasks

            def front_head(b, dc, h):
                kt, qt = kT[b][dc], qT[b][dc]
                po = h * 64
                sc = psABC.tile([128, 3, SE], F32, name="sc", tag="scp",
                                padded_shape=[128, 3, 512], bufs=1)
                for sj in range(3):
                    nc.tensor.matmul(sc[:, sj, :],
                                     kt[po:po + 64, sj * 128:(sj + 1) * 128],
                                     qt[po:po + 64, :],
                                     start=True, stop=sj != 2)
                nc.tensor.matmul(sc[:, 2, :], padneg[:], ones_row[:],
                                 start=False, stop=True)
                pT = sbC.tile([128, 3, SE], F16, name="pT", tag="pT", bufs=4)
                nc.scalar.activation(pT[:], sc[:, :, :], AF.Exp)
                return pT

            def back(b, dc, pTA, pTB):
                cs = psABC.tile([128, SE], F32, name="cs", tag="csr", bufs=1)
                for sj in range(3):
                    nc.tensor.matmul(cs[0:64, :], onesb[:, :], pTA[:, sj, :],
                                     start=sj == 0, stop=sj == 2)
                    nc.tensor.matmul(cs[64:128, :], onesb[:, :], pTB[:, sj, :],
                                     start=sj == 0, stop=sj == 2)
                rcb = sbC.tile([128, SE], F32, name="rcb", tag="rcb", bufs=2)
                nc.vector.reciprocal_approx_fast(rcb[:], cs[:])
                at = psABC.tile([128, SE], F32, name="at", tag="att", bufs=1)
                for sj in range(3):
                    nc.tensor.matmul(at[0:64, :],
                                     vna[b][sj][:, dc * 128:dc * 128 + 64],
                                     pTA[:, sj, :],
                                     start=sj == 0, stop=sj == 2)
                    nc.tensor.matmul(
                        at[64:128, :],
                        vna[b][sj][:, dc * 128 + 64:(dc + 1) * 128],
                        pTB[:, sj, :], start=sj == 0, stop=sj == 2)
                t = attnTp.tile([128, SE], F16, name="atT", tag="atT")
                nc.vector.tensor_tensor(out=t[:], in0=at[:], in1=rcb[:],
                                        op=ALU.mult)
                attnT[b][dc] = t

            # stage A interleaved with b0's qkv (prologue)
            stage_a(0)
            w_loads["q"] = load_w(qw_d)
            w_loads["k"] = load_w(kw_d)
            prolog = deque(proj_tasks(0))
            for b in range(1, NB):
                stage_a(b)
                if b == 1:
                    w_loads["v"] = load_w(vw_d)
                for _ in range(8):
                    if prolog:
                        prolog.popleft()()
            while prolog:
                prolog.popleft()()
            w_loads["o"] = load_w(ow_d)
            esA.close()

            esC = ExitStack()
            sbC = esC.enter_context(tc.tile_pool(name="sbC", bufs=2))

            filler = deque()
            pend = None
            for b in range(NB):
                if b + 1 < NB:
                    filler.extend(proj_tasks(b + 1))
                else:
                    for ob in (0, 1):
                        filler.extend([lambda ob=ob, mc=mc: o_mc(ob, mc)
                                       for mc in range(DC)])
                for dc in range(DC):
                    pTA = front_head(b, dc, 0)
                    for _ in range(2):
                        if filler:
                            filler.popleft()()
                    pTB = front_head(b, dc, 1)
                    for _ in range(2):
                        if filler:
                            filler.popleft()()
                    if pend is not None:
                        back(*pend)
                    pend = (b, dc, pTA, pTB)
                while filler:
                    filler.popleft()()
            back(*pend)
            esC.close()

            # out_proj for b2 (b3's is interleaved into stage D below)
            for mc in range(DC):
                o_mc(2, mc)
            esPS.close()   # psABC dead
            esA_HT.close()  # HT dead
            esBC.close()   # qT, kT, vna dead

            # ---------- stage D: residual, LN2, h2^T ----------
            esD = ExitStack()
            psD = esD.enter_context(
                tc.tile_pool(name="psD", bufs=6, space="PSUM"))
            esDE = ExitStack()
            H2Tp = esDE.enter_context(
                tc.tile_pool(name="H2T", bufs=NB * DC, side="right"))
            x1P = esDE.enter_context(
                tc.tile_pool(name="x1P", bufs=NB * 3, side="right"))
            esD3 = ExitStack()
            h2natp = esD3.enter_context(
                tc.tile_pool(name="h2nat", bufs=6, side="right"))
            H2T = [None] * NB
            x1 = []
            h2nat_all = [None] * NB

            def stage_d1(b):
                x1ts = []
                for j, (o, pz) in enumerate(SQ):
                    xres = xio.tile([pz, D], F32, name="xres", tag="xin")
                    nc.sync.dma_start(xres[:], x_d[b, o:o + pz, :])
                    x1t = x1P.tile([pz, D], F16, name="x1", tag="x1")
                    for hf in range(2):
                        ps = psD.tile([pz, 512], F16, name="psN", tag="pp")
                        for dl in range(4):
                            dc = hf * 4 + dl
                            nc.tensor.transpose(
                                ps[:, dl * 128:(dl + 1) * 128],
                                aoT[b][dc][:, o:o + pz], ident16[:128, :128])
                        nc.vector.tensor_tensor(
                            out=x1t[:, hf * 512:(hf + 1) * 512], in0=ps[:],
                            in1=xres[:, hf * 512:(hf + 1) * 512], op=ALU.add)
                    x1ts.append(x1t)
                h2nat_all[b] = layer_norm(x1ts, h2natp)
                x1.append(x1ts)

            stage_d1(0)
            ow_left = deque(range(DC))
            for _ in range(3):
                o_mc(3, ow_left.popleft(), psD)
            for b in range(1, NB):
                stage_d1(b)
                for _ in range(3):
                    if ow_left:
                        o_mc(3, ow_left.popleft(), psD)
                H2T[b - 1] = transpose_to_T(h2nat_all[b - 1], H2Tp, g2_sb,
                                            b2_sb, "H2T", psD)
            H2T[NB - 1] = transpose_to_T(h2nat_all[NB - 1], H2Tp, g2_sb,
                                         b2_sb, "H2T", psD)
            esD2.close()  # aoT dead
            esW.close()   # weight pool dead
            esD3.close()  # h2nat dead
            esD.close()   # psD dead

            # ---------- stage E: MLP, single pass ----------
            esE = ExitStack()
            psE = esE.enter_context(
                tc.tile_pool(name="psE", bufs=6, space="PSUM"))
            w1p = esE.enter_context(tc.tile_pool(name="w1", bufs=3))
            w2p = esE.enter_context(tc.tile_pool(name="w2", bufs=2))
            h1Tp = esE.enter_context(tc.tile_pool(name="h1T", bufs=NB * FC))
            moTp = esE.enter_context(tc.tile_pool(name="moT", bufs=NB * DC))
            outnp = esE.enter_context(tc.tile_pool(name="outn", bufs=2))
            h1T = [[None] * FC for _ in range(NB)]
            for mc in range(FC):
                w1t = w1p.tile([128, DC, 128], F16, name="w1", tag="w1")
                nc.sync.dma_start(w1t[:], f1w_d[:, mc])
                for b in range(NB):
                    ps = psE.tile([128, SE], F32, name="psF1", tag="pp")
                    for kc in range(DC):
                        nc.tensor.matmul(
                            ps[:], w1t[:, kc, :], H2T[b][kc][:],
                            start=(kc == 0), stop=(kc == DC - 1))
                    t = h1Tp.tile([128, SE], F16, name="h1T", tag="h1T")
                    nc.scalar.activation(t[:], ps[:], AF.Gelu_apprx_sigmoid,
                                         bias=f1b_sb[:, mc:mc + 1])
                    h1T[b][mc] = t
            moT = [[None] * DC for _ in range(NB)]
            for mc in range(DC):
                w2t = w2p.tile([128, FC, 128], F16, name="w2", tag="w2")
                nc.sync.dma_start(w2t[:], f2w_d[:, mc])
                for b in range(NB):
                    ps = psE.tile([128, SE], F32, name="psF2", tag="pp")
                    for kc in range(FC):
                        nc.tensor.matmul(
                            ps[:], w2t[:, kc, :], h1T[b][kc][:],
                            start=(kc == 0), stop=(kc == FC - 1))
                    t = moTp.tile([128, SE], F16, name="moT", tag="moT")
                    nc.vector.tensor_scalar_add(t[:], ps[:],
                                                f2b_sb[:, mc:mc + 1])
                    moT[b][mc] = t
            for b in range(NB):
                for j, (o, pz) in enumerate(SQ):
                    ot = outnp.tile([pz, D], F32, name="outn", tag="outn")
                    for hf in range(2):
                        ps = psE.tile([pz, 512], F16, name="psO", tag="pp")
                        for dl in range(4):
                            dc = hf * 4 + dl
                            nc.tensor.transpose(
                                ps[:, dl * 128:(dl + 1) * 128],
                                moT[b][dc][:, o:o + pz], ident16[:128, :128])
                        nc.vector.tensor_tensor(
                            out=ot[:, hf * 512:(hf + 1) * 512], in0=ps[:],
                            in1=x1[b][j][:, hf * 512:(hf + 1) * 512],
                            op=ALU.add)
                    nc.sync.dma_start(out_d[b, o:o + pz, :], ot[:])
            esE.close()
            esDE.close()
            esCD.close()   # attnT dead

    nc.compile()
    return nc


_NC = None


def _get_nc():
    global _NC
    if _NC is None:
        _NC = build()
    return _NC


def _prep_inputs(inputs):
    f = lambda a: np.ascontiguousarray(np.asarray(a, dtype=np.float32))
    x = f(inputs["hidden_states"])

    def packw(W, scale=1.0):
        # W: [dout, din] -> [128, DC, dout] where [p, kc, m] = W[m, kc*128+p]
        A = (f(W).T * scale).astype(np.float16)  # [din, dout]
        return np.ascontiguousarray(
            A.reshape(DC, 128, -1).transpose(1, 0, 2))

    def packf(W, nin_c, nout_c):
        # W: [dout, din] -> [128, nout_c, nin_c, 128]
        A = f(W).T.astype(np.float16)  # [din, dout]
        return np.ascontiguousarray(
            A.reshape(nin_c, 128, nout_c, 128).transpose(1, 2, 0, 3))

    shared = {
        "qw": packw(inputs["q_w"], SCALE),
        "kw": packw(inputs["k_w"]),
        "vw": packw(inputs["v_w"]),
        "ow": packw(inputs["o_w"]),
        "f1w": packf(inputs["fc1_w"], DC, FC),
        "f2w": packf(inputs["fc2_w"], FC, DC),
        "qb": f(inputs["q_b"]) * SCALE,
        "kb": f(inputs["k_b"]),
        "ob": f(inputs["o_b"]) + f(inputs["o_w"]) @ f(inputs["v_b"]),
        "f1b": f(inputs["fc1_b"]),
        "f2b": f(inputs["fc2_b"]),
        "g1": f(inputs["ln1_g"]),
        "b1": f(inputs["ln1_b"]),
        "g2": f(inputs["ln2_g"]),
        "b2": f(inputs["ln2_b"]),
    }
    shared = {k: np.ascontiguousarray(v) for k, v in shared.items()}
    in_maps = []
    for c in range(N_CORES):
        m = dict(shared)
        m["x"] = np.ascontiguousarray(x[c * NB:(c + 1) * NB])
        in_maps.append(m)
    return in_maps


def run(inputs, trace=False):
    nc = _get_nc()
    in_maps = _prep_inputs(inputs)
    res = bass_utils.run_bass_kernel_spmd(
        nc, in_maps, core_ids=list(range(N_CORES)), trace=trace)
    out = np.concatenate([res.results[c]["out"] for c in range(N_CORES)],
                         axis=0)
    return out, res


def kernel(**inputs):
    out, _ = run(inputs, trace=False)
    return out
